# revision 52
# baseline (speedup 1.0000x reference)
"""Trainium2 Bass kernel for nn_CalibrationNetwork (MoE routing over 12 judges).

Strategy: shared + judge-specific weights are pre-summed on the host into 12
effective per-judge MLPs; samples are sorted by judge id, each judge's slots
padded to capacity 2*Cc (Cc=1408, n-tiles 512/512/384 — sized to the actual
max judge count), and the 24 chunks dealt 3-per-core to 8 NeuronCores running
one static Bass/Tile program.

PE-array concurrency (tile_position row/col tiling) carries the slot count:
layer1 (K=36) runs as 3 row-tiled PAIRS — two matmuls with stationaries at
array rows 0-35 / 64-99 and moving operands at the matching partition rows
issue back-to-back and overlap in the array, so 6 matmuls cost 3 slots.  The
n2 x-block is duplicated at rows 0-35 and 64-99 so every pair is balanced.
The per-question heads (M=36) are col-tiled: n0's logits land at PSUM
partitions 0-35 (array cols 0-35) while n1's land at partitions 64-99 (cols
64-99), halving head slots.  Layer2 uses the full 128x128 array (no tiling).

Relu/bias evacuations are spread over ACT, DVE and GPSIMD so no single
engine exceeds the PE's ~4.1us/chunk.  Inputs arrive as two merged DMAs per
chunk (l1 inputs [100,1152] on sync, weights+heads [128,584] on gpsimd), and
each packed chunk's softmax output leaves as ONE [108,512] DMA against an
out DRAM layout of [108, SEG, 512] (row 36b+o = n-block b, question-out o).

Softmax head-major: heads accumulate a chunk's logits in a 2-bank PSUM tile;
exp(logit+bias) on ACT (pad rows get bias -1e30 -> e=0); the three 36-row
blocks are DMA-packed into one lane-dense [108,512] tile, then ONE group-sums
matmul, ONE reciprocal, ONE f32r cast, ONE broadcast matmul, ONE multiply.
The last chunk runs an unpacked tail (no pack-DMA latency): col-tiled n0/n1
plus the 384-col n2 split into two 192-col pieces, each piece a short
exp->sums->recip->cast->bcast->mult->DMA chain staggered across engines.

The PE clock gate (HAM) needs ~3.4us of busy to reach 2.4 GHz: 4 fp32
warmup matmuls (853ns each cold) bridge the initial DMA wait, and chunk s's
softmax matmuls are emitted between chunk s+1's layer2 m-phases to keep the
in-order PE queue dense.
"""

import os
import sys

import ml_dtypes
import numpy as np

for _p in ("/opt/trn_rl_repo", "/root/.axon_site/_ro/trn_rl_repo"):
    if os.path.isdir(_p) and _p not in sys.path:
        sys.path.insert(0, _p)

B, D, H1, H2, J, Q, O = 32768, 35, 256, 256, 12, 7, 5
NCORES = 8
SEG = 3                    # chunks per core
NCHUNKS = NCORES * SEG     # 24 = 2 chunks per judge
QO = Q * O                 # 35
QOp = QO + 1               # padded head dim

USE_F32R = True            # PE fast-fp32 mode (1 cyc/col vs 4 for fp32)
WARMUP_MM = 4              # fp32 warmup matmuls
WARM_N = 256
LDW_OPT = False            # walrus LDWEIGHTS optimizer (rejects col-tiled LDW)
TRACE = False
LAST_RESULTS = None

_PROG_CACHE = {}


def _patch_ldw_opt():
    """enable the LDWEIGHTS optimizer in walrus (overlaps/dedups stationary
    loads; measured ~11% end-to-end, bit-identical outputs)."""
    from concourse import bass_utils as BU

    if not LDW_OPT or getattr(BU, "_ldw_opt_patched", False):
        return
    orig = BU.run_command

    def patched(argv, **kw):
        argv = [
            "--enable-ldw-opt=true" if a == "--enable-ldw-opt=false" else a
            for a in argv
        ]
        return orig(argv, **kw)

    BU.run_command = patched
    BU._ldw_opt_patched = True


def _build_program(Cc, use_f32r):
    import contextlib

    import concourse.tile as tile
    from concourse import bacc, mybir

    f32 = mybir.dt.float32
    fmm = mybir.dt.float32r if use_f32r else f32
    bf16 = mybir.dt.bfloat16
    AF = mybir.ActivationFunctionType
    ALU = mybir.AluOpType

    W3 = Cc - 1024             # width of the third n-tile
    assert 128 <= W3 <= 512 and W3 % 64 == 0
    offs = (0, 512, 1024)
    widths = (512, 512, W3)
    A1OFF = 512 + W3           # a1 columns inside the merged l1in tile

    nc = bacc.Bacc(None, target_bir_lowering=False, debug=False, num_swdge_queues=2)

    l1in_d = nc.dram_tensor("l1in", [SEG, 100, A1OFF + H1], bf16, kind="ExternalInput")
    a2av_d = nc.dram_tensor("a2av", [SEG, 128, 512 + 2 * QOp], bf16, kind="ExternalInput")
    bias_d = nc.dram_tensor("biasall", [128, 3 * SEG], f32, kind="ExternalInput")
    # onesb columns: [0:24] packed sums, [24:132] packed bcast,
    # [132:140] single sums (rows 0-35), [140:176] single bcast (rows 0-7),
    # [176:184] single sums (rows 64-99), [184:312] ones (epk pad source)
    ones_d = nc.dram_tensor("onesb", [108, 312], fmm, kind="ExternalInput")
    # fp32 copies of the bcast matrices: the broadcast matmul runs in plain
    # fp32 so the reciprocal feeds it directly (no f32r rounding-copy op)
    onesf_d = nc.dram_tensor("onesf", [24, 144], f32, kind="ExternalInput")
    out_d = nc.dram_tensor("out", [108, SEG * 512], bf16, kind="ExternalOutput")

    lp = (
        nc.allow_low_precision(reason="float32r matmul operands are intentional")
        if use_f32r
        else contextlib.nullcontext()
    )
    with lp, tile.TileContext(nc) as tc:
        with (
            tc.tile_pool(name="xp", bufs=1) as xp,        # constants / warmup
            tc.tile_pool(name="inp", bufs=2) as inp,      # per-chunk inputs
            tc.tile_pool(name="zp", bufs=2) as zp,        # z1 / z2
            tc.tile_pool(name="op", bufs=2) as op_,       # softmax SBUF tiles
            tc.tile_pool(name="psM", bufs=5, space="PSUM") as psM,    # 5 banks
            tc.tile_pool(name="psH", bufs=1, space="PSUM") as psH,    # 2 banks
            tc.tile_pool(name="psS", bufs=1, space="PSUM") as psS,    # 1 bank
        ):
            onesb = xp.tile([108, 312], fmm)
            onesf = xp.tile([24, 144], f32)
            biasall = xp.tile([128, 3 * SEG], f32)
            ones_s = onesb[0:108, 0:24]
            ones_s1 = onesb[0:QOp, 132:140]
            ones_r = onesf[0:24, 0:108]
            ones_r1 = onesf[0:8, 108:144]

            def emit_load(s):
                h = {}
                # layout [a1 | xta | xtc]; two transfers so the first (a1+xta,
                # all three pair-A/C operands) lands before xtc is needed
                l1 = inp.tile([100, A1OFF + H1], bf16, tag="l1in", name=f"l1in_{s}")
                nc.sync.dma_start(l1[:, 0 : H1 + 512], l1in_d[s][:, 0 : H1 + 512])
                nc.sync.dma_start(l1[:, H1 + 512 :], l1in_d[s][:, H1 + 512 :])
                a2av = inp.tile([128, 512 + 2 * QOp], bf16, tag="a2av", name=f"a2av_{s}")
                nc.gpsimd.dma_start(a2av[:], a2av_d[s])
                h["a1"] = l1[:, 0:H1]
                h["xta"] = l1[:, H1 : H1 + 512]
                h["xtc"] = l1[:, H1 + 512 : H1 + 512 + W3]
                h["a2av"] = a2av
                h["z1"] = zp.tile([128, 2, Cc], bf16, tag="z1", name=f"z1_{s}")
                h["z2"] = zp.tile([128, 2, Cc], bf16, tag="z2", name=f"z2_{s}")
                return h

            def evac(eng, dst, src, bias=None):
                """PSUM->SBUF relu evacuation on ACT / DVE / GPSIMD."""
                if eng == "act":
                    if bias is None:
                        nc.scalar.activation(dst, src, AF.Relu)
                    else:
                        nc.scalar.activation(dst, src, AF.Relu, bias=bias)
                    return
                e = nc.vector if eng == "dve" else nc.gpsimd
                if bias is None:
                    e.tensor_scalar(
                        out=dst, in0=src, scalar1=0.0, scalar2=None, op0=ALU.max
                    )
                else:
                    e.tensor_scalar(
                        out=dst, in0=src, scalar1=bias, scalar2=0.0,
                        op0=ALU.add, op1=ALU.max,
                    )

            def emit_l1(s, h):
                """3 row-tiled pair slots: stationaries at array rows 0-35 and
                64-99 with moving operands at matching partitions overlap.
                Pair and evacuation-engine order matches layer2's consumption
                order (k0n0, k0n1, k1n0, k1n1, k0n2, k1n2) so each engine's
                FIFO delivers z1 halves exactly as layer2 needs them."""
                a1, xta, xtc, z1 = h["a1"], h["xta"], h["xtc"], h["z1"]
                pairs = [
                    # (w, lhsT_lo, rhs_lo, dst_lo, eng_lo, lhsT_hi, rhs_hi, dst_hi, eng_hi)
                    (512, a1[0:QOp, 0:128], xta[0:QOp, :], z1[:, 0, 0:512], "act",
                     a1[64:64 + QOp, 0:128], xta[64:64 + QOp, :], z1[:, 0, 512:1024], "dve"),
                    (512, a1[0:QOp, 128:256], xta[0:QOp, :], z1[:, 1, 0:512], "dve",
                     a1[64:64 + QOp, 128:256], xta[64:64 + QOp, :], z1[:, 1, 512:1024], "act"),
                    (W3, a1[0:QOp, 0:128], xtc[0:QOp, :], z1[:, 0, 1024:Cc], "act",
                     a1[64:64 + QOp, 128:256], xtc[64:64 + QOp, :], z1[:, 1, 1024:Cc], "dve"),
                ]
                for i, (w, llo, rlo, dlo, elo, lhi, rhi, dhi, ehi) in enumerate(pairs):
                    plo = psM.tile([128, 512], f32, tag="mm", name=f"p1_{s}{i}a")
                    phi = psM.tile([128, 512], f32, tag="mm", name=f"p1_{s}{i}b")
                    nc.tensor.matmul(plo[:, 0:w], llo, rlo, start=True, stop=True)
                    nc.tensor.matmul(phi[:, 0:w], lhi, rhi, start=True, stop=True)
                    evac(elo, dlo, plo[:, 0:w])
                    evac(ehi, dhi, phi[:, 0:w])

            # engine per (m, n): parallel ACT/DVE pairs in heads-consumption
            # order (heads k0 pair needs m0-n0/n1, k1 pair needs m1-n0/n1)
            L2_ENG = {(0, 0): "act", (0, 1): "dve", (0, 2): "act",
                      (1, 0): "dve", (1, 1): "act", (1, 2): "dve"}

            def emit_l2(s, h, m):
                """n0/n1 first (both k passes), n2 last — matches the arrival
                order of z1 evacuations and heads' consumption of z2."""
                a2av, z1, z2 = h["a2av"], h["z1"], h["z2"]
                bias = biasall[:, 3 * s + m : 3 * s + m + 1]
                st = [a2av[:, k * 256 + m * 128 : k * 256 + (m + 1) * 128]
                      for k in range(2)]
                p0 = psM.tile([128, 512], f32, tag="mm", name=f"p2_{s}{m}0")
                p1 = psM.tile([128, 512], f32, tag="mm", name=f"p2_{s}{m}1")
                nc.tensor.matmul(p0[:], st[0], z1[:, 0, 0:512], start=True, stop=False)
                nc.tensor.matmul(p1[:], st[0], z1[:, 0, 512:1024], start=True, stop=False)
                nc.tensor.matmul(p0[:], st[1], z1[:, 1, 0:512], start=False, stop=True)
                evac(L2_ENG[(m, 0)], z2[:, m, 0:512], p0[:], bias=bias)
                nc.tensor.matmul(p1[:], st[1], z1[:, 1, 512:1024], start=False, stop=True)
                evac(L2_ENG[(m, 1)], z2[:, m, 512:1024], p1[:], bias=bias)
                p2 = psM.tile([128, 512], f32, tag="mm", name=f"p2_{s}{m}2")
                nc.tensor.matmul(p2[:, 0:W3], st[0], z1[:, 0, 1024:Cc],
                                 start=True, stop=False)
                nc.tensor.matmul(p2[:, 0:W3], st[1], z1[:, 1, 1024:Cc],
                                 start=False, stop=True)
                evac(L2_ENG[(m, 2)], z2[:, m, 1024:Cc], p2[:, 0:W3], bias=bias)

            def emit_heads(s, h, last=False):
                """Col-tiled heads: n0 -> PSUM partitions 0-35 (array cols
                0-35), n1 -> partitions 64-99 (cols 64-99) — the two matmuls
                of each k overlap in the array.  n2 runs alone at (0,0)."""
                a2av, z2 = h["a2av"], h["z2"]
                blo = biasall[0:QOp, 3 * s + 2 : 3 * s + 3]
                bhi = biasall[64:64 + QOp, 3 * s + 2 : 3 * s + 3]
                if last:
                    # tail chunk: all logits at partitions 0-35 (f32r softmax
                    # matmuls cannot col-tile); n0+n2 share the psH tile so
                    # the tail's sums/bcast matmuls get the psM pool to
                    # themselves, n1 takes one psM tile
                    el = op_.tile([QOp, Cc], fmm, tag="el", name=f"el_{s}")
                    pha = psH.tile([100, 512 + W3], f32, tag="ph", name=f"phl_{s}a")
                    phb = psM.tile([QOp, 512], f32, tag="mm", name=f"phl_{s}b")
                    for k in range(2):
                        avk = a2av[:, 512 + k * QOp : 512 + (k + 1) * QOp]
                        nc.tensor.matmul(pha[0:QOp, 0:512], avk, z2[:, k, 0:512],
                                         start=(k == 0), stop=(k == 1))
                        nc.tensor.matmul(phb[0:QOp, 0:512], avk, z2[:, k, 512:1024],
                                         start=(k == 0), stop=(k == 1))
                        nc.tensor.matmul(pha[0:QOp, 512:512 + W3], avk,
                                         z2[:, k, 1024:Cc],
                                         start=(k == 0), stop=(k == 1))
                    wp = W3 // 2
                    nc.scalar.activation(el[:, 0:512], pha[0:QOp, 0:512],
                                         AF.Exp, bias=blo)
                    nc.scalar.activation(el[:, 512:1024], phb[0:QOp, 0:512],
                                         AF.Exp, bias=blo)
                    nc.scalar.activation(el[:, 1024:1024 + wp],
                                         pha[0:QOp, 512:512 + wp],
                                         AF.Exp, bias=blo)
                    nc.scalar.activation(el[:, 1024 + wp:Cc],
                                         pha[0:QOp, 512 + wp:512 + W3],
                                         AF.Exp, bias=blo)
                    return None, el, None
                ph = psH.tile([100, 512 + W3], f32, tag="ph", name=f"ph_{s}")
                # k0 pair, n2-k0, k1 pair, n2-k1: the n2-k0 matmul fills the
                # wait for z2[m1] evacuations before the k1 pair
                for k in range(2):
                    avk = a2av[:, 512 + k * QOp : 512 + (k + 1) * QOp]
                    nc.tensor.matmul(ph[0:QOp, 0:512], avk, z2[:, k, 0:512],
                                     start=(k == 0), stop=(k == 1))
                    nc.tensor.matmul(ph[64:64 + QOp, 0:512], avk, z2[:, k, 512:1024],
                                     start=(k == 0), stop=(k == 1))
                    nc.tensor.matmul(ph[0:QOp, 512:512 + W3], avk, z2[:, k, 1024:Cc],
                                     start=(k == 0), stop=(k == 1))
                e = op_.tile([100, 512 + W3], fmm, tag="e", name=f"e_{s}")
                epk = op_.tile([QOp * 3, 512], fmm, tag="epk", name=f"epk_{s}")
                nc.scalar.activation(epk[0:QOp, 0:512], ph[0:QOp, 0:512],
                                     AF.Exp, bias=blo)
                nc.scalar.activation(e[64:64 + QOp, 0:512], ph[64:64 + QOp, 0:512],
                                     AF.Exp, bias=bhi)
                nc.sync.dma_start(epk[QOp : 2 * QOp, 0:512], e[64:64 + QOp, 0:512])
                nc.scalar.activation(e[0:QOp, 512:512 + W3], ph[0:QOp, 512:512 + W3],
                                     AF.Exp, bias=blo)
                nc.gpsimd.dma_start(epk[2 * QOp : 3 * QOp, 0:W3],
                                    e[0:QOp, 512:512 + W3])
                if W3 < 512:
                    # fill the pad columns with 1.0: keeps their group sums
                    # nonzero (a 0-sum -> Inf recip -> 0*Inf NaN would pollute
                    # every row of those columns through the bcast matmul)
                    nc.gpsimd.dma_start(epk[2 * QOp : 3 * QOp, W3:512],
                                        ones_d[2 * QOp : 3 * QOp, 184 : 184 + 512 - W3])
                return ph, e, epk

            def emit_sm_sums(s, epk):
                sm = psS.tile([24, 512], f32, tag="smbc", name=f"sm_{s}")
                nc.tensor.matmul(sm[:], ones_s, epk[:], start=True, stop=True)
                rt = op_.tile([24, 512], f32, tag="rt", name=f"rt_{s}")
                nc.vector.reciprocal_approx_fast(rt[:], sm[:])
                return rt

            def emit_sm_fin(s, epk, rt):
                # broadcast matmul in plain fp32 (4 cyc/col): eats the f32r
                # rounding-copy op and feeds from the reciprocal directly
                bc = psS.tile([108, 512], f32, tag="smbc", name=f"bc_{s}")
                nc.tensor.matmul(bc[:], ones_r, rt[:], start=True, stop=True)
                outm = op_.tile([108, 512], bf16, tag="om", name=f"om_{s}")
                nc.vector.tensor_tensor(outm[:], epk[:], bc[:], ALU.mult)
                nc.sync.dma_start(out_d[:, s * 512 : (s + 1) * 512], outm[:])

            def emit_tail_last(s, el):
                """Unpacked per-piece softmax tail for the final chunk: four
                pieces (512/512/W3÷2/W3÷2), all at partitions 0-35."""
                wp = W3 // 2
                # pieces: (el col, width, out block row, out col)
                P = [
                    (0, 512, 0, 0),
                    (512, 512, QOp, 0),
                    (1024, wp, 2 * QOp, 0),
                    (1024 + wp, wp, 2 * QOp, wp),
                ]
                rt = op_.tile([8, Cc], f32, tag="rtl", name=f"rtl_{s}")
                outm = op_.tile([QOp, Cc], bf16, tag="oml", name=f"oml_{s}")
                for i, (ec, w, ob, oc) in enumerate(P):
                    sm = psM.tile([8, 512], f32, tag="mm", name=f"sml_{s}{i}")
                    nc.tensor.matmul(sm[:, 0:w], ones_s1, el[:, ec:ec + w],
                                     start=True, stop=True)
                    nc.vector.reciprocal_approx_fast(rt[:, ec:ec + w], sm[:, 0:w])
                for i, (ec, w, ob, oc) in enumerate(P):
                    bc = psM.tile([QOp, 512], f32, tag="mm", name=f"bcl_{s}{i}")
                    nc.tensor.matmul(bc[0:QOp, 0:w], ones_r1, rt[:, ec:ec + w],
                                     start=True, stop=True)
                    nc.vector.tensor_tensor(outm[:, ec:ec + w],
                                            el[:, ec:ec + w], bc[0:QOp, 0:w],
                                            ALU.mult)
                    deng = (nc.gpsimd, nc.sync)[i % 2]
                    deng.dma_start(out_d[ob:ob + QOp, s * 512 + oc : s * 512 + oc + w],
                                   outm[:, ec:ec + w])

            # ---- program body ----
            h0 = emit_load(0)
            nc.sync.dma_start(onesb[:], ones_d[:])
            nc.gpsimd.dma_start(biasall[:], bias_d[:])
            nc.gpsimd.dma_start(onesf[:], onesf_d[:])
            h1 = emit_load(1)

            # PE warmup: fp32 matmuls (4 cyc/col) bridge the initial DMA wait
            # and start the HAM activity window.
            wsrc = xp.tile([128, WARM_N], f32, tag="warmsrc")
            nc.vector.memset(wsrc[:], 0.0)
            wps = psS.tile([128, 512], f32, tag="smbc", name="warm_ps")
            for _ in range(WARMUP_MM):
                nc.tensor.matmul(wps[:, 0:WARM_N], wsrc[:, 0:128], wsrc[:],
                                 start=True, stop=True)

            # software pipeline: chunk s+1's layer1 is emitted BEFORE chunk
            # s's heads so the PE has independent work while z2(s)
            # evacuations drain; chunk s's softmax matmuls slot between
            # chunk s+1's layer2 m-phases.
            emit_l1(0, h0)
            emit_l2(0, h0, 0)
            emit_l2(0, h0, 1)
            h2 = emit_load(2)
            hs = [h0, h1, h2]
            emit_l1(1, h1)
            pend = (0, emit_heads(0, h0))
            for s in range(1, SEG):
                hn = hs[s]
                emit_l2(s, hn, 0)
                ps, (pph, pe, pepk) = pend
                rt = emit_sm_sums(ps, pepk)
                emit_l2(s, hn, 1)
                emit_sm_fin(ps, pepk, rt)
                if s < SEG - 1:
                    emit_l1(s + 1, hs[s + 1])
                pend = (s, emit_heads(s, hn, last=(s == SEG - 1)))
            ps, (pph, pe, pepk) = pend
            emit_tail_last(ps, pe)

    nc.compile()
    return nc


def _get_program(Cc, use_f32r):
    key = (Cc, use_f32r)
    if key not in _PROG_CACHE:
        _PROG_CACHE[key] = _build_program(Cc, use_f32r)
    return _PROG_CACHE[key]


def kernel(**inputs):
    global LAST_RESULTS
    x = np.ascontiguousarray(np.asarray(inputs["x"], dtype=np.float32))
    ids = np.asarray(inputs["judge_ids"]).astype(np.int64).ravel()
    W1_w = np.asarray(inputs["W1_w"], np.float32)
    W1_b = np.asarray(inputs["W1_b"], np.float32)
    W2_w = np.asarray(inputs["W2_w"], np.float32)
    W2_b = np.asarray(inputs["W2_b"], np.float32)
    W1a_w = np.asarray(inputs["W1a_w"], np.float32)
    W1a_b = np.asarray(inputs["W1a_b"], np.float32)
    W2a_w = np.asarray(inputs["W2a_w"], np.float32)
    W2a_b = np.asarray(inputs["W2a_b"], np.float32)
    V_w = np.asarray(inputs["V_w"], np.float32)
    V_b = np.asarray(inputs["V_b"], np.float32)
    Va_w = np.asarray(inputs["Va_w"], np.float32)
    Va_b = np.asarray(inputs["Va_b"], np.float32)

    Bx = x.shape[0]
    cnts = np.bincount(ids, minlength=J)
    Cc = 1408
    mx = int(cnts.max())
    while 2 * Cc < mx and Cc < 1536:
        Cc += 128
    assert 2 * Cc >= mx, "judge capacity exceeded"
    W3 = Cc - 1024
    offs = (0, 512, 1024)
    widths = (512, 512, W3)
    A1OFF = 512 + W3

    # effective per-judge weights (shared + judge-specific, biases folded)
    A1 = (W1_w[None] + W1a_w).copy()                      # (J, H1, D+1)
    A1[:, :, D] += W1_b[None] + W1a_b
    A2 = W2_w[None] + W2a_w                               # (J, H2, H1+1)
    b2 = A2[:, :, H1] + W2_b[None] + W2a_b                # (J, H2)
    A2c = A2[:, :, :H1]                                   # (J, H2, H1)
    AV = (V_w[None] + Va_w).reshape(J, QO, H2 + 1)
    bV = (AV[:, :, H2] + (V_b[None] + Va_b).reshape(J, QO)).astype(np.float32)
    AVc = AV[:, :, :H2]

    # SBUF layouts
    a1sb = np.ascontiguousarray(np.transpose(A1, (0, 2, 1)))  # (J, 36, 256)
    a2sb = np.transpose(A2c.reshape(J, H2, 2, 128), (0, 3, 2, 1))
    # (J, 128, 2, 256): [j,p,k,m] = A2c[j][m, k*128+p]
    avsb = np.transpose(AVc.reshape(J, QO, 2, 128), (0, 3, 2, 1))  # (J,128,2,35)
    avsb = np.concatenate(
        [avsb, np.zeros((J, 128, 2, QOp - QO), np.float32)], axis=3
    )
    a2av = np.concatenate(
        [
            np.ascontiguousarray(a2sb).reshape(J, 128, 512),
            np.ascontiguousarray(avsb).reshape(J, 128, 2 * QOp),
        ],
        axis=2,
    )  # (J, 128, 584)
    b2sb = np.ascontiguousarray(np.transpose(b2.reshape(J, 2, 128), (0, 2, 1)))

    # softmax bias column: rows 0-34 bV, rows 64-98 bV (for col-tiled n1),
    # all other rows -1e30 (pad -> e = 0)
    biasc = np.full((J, 128, 1), -1e30, np.float32)
    biasc[:, 0:QO, 0] = bV
    biasc[:, 64:64 + QO, 0] = bV
    bias3 = np.concatenate([b2sb, biasc], axis=2)  # (J, 128, 3)

    # ones matrices: packed (3-block) + single-block lo/hi + zero pad block
    onesb = np.zeros((108, 312), np.float32)
    for b in range(3):
        for o_ in range(QO):
            q = o_ // O
            onesb[QOp * b + o_, 8 * b + q] = 1.0
            onesb[8 * b + q, 24 + QOp * b + o_] = 1.0
        onesb[QOp * b : QOp * (b + 1), 8 * b + 7] = 1.0
    for o_ in range(QO):
        q = o_ // O
        onesb[o_, 132 + q] = 1.0
        onesb[q, 140 + o_] = 1.0
        onesb[64 + o_, 176 + q] = 1.0
    onesb[0:QOp, 132 + 7] = 1.0
    onesb[64:64 + QOp, 176 + 7] = 1.0
    onesb[:, 184:312] = 1.0          # epk pad fill (keeps pad sums nonzero)
    onesb = np.ascontiguousarray(onesb)
    # fp32 bcast matrices: [0:108] packed (24 rows), [108:144] single (8 rows)
    onesf = np.zeros((24, 144), np.float32)
    for b in range(3):
        for o_ in range(QO):
            onesf[8 * b + o_ // O, QOp * b + o_] = 1.0
    for o_ in range(QO):
        onesf[o_ // O, 108 + o_] = 1.0
    onesf = np.ascontiguousarray(onesf)

    # slot -> sample map: judge j owns slots [j*2Cc, (j+1)*2Cc)
    order = np.argsort(ids, kind="stable")
    slot2samp = np.full(NCHUNKS * Cc, -1, np.int64)
    pos = 0
    for j in range(J):
        k = int(cnts[j])
        slot2samp[j * 2 * Cc : j * 2 * Cc + k] = order[pos : pos + k]
        pos += k
    chunk_judge = np.repeat(np.arange(J), 2)

    in_maps = []
    core_meta = []
    for c in range(NCORES):
        sl = slot2samp[c * SEG * Cc : (c + 1) * SEG * Cc]
        valid = sl >= 0
        Xc = np.zeros((SEG * Cc, D + 1), np.float32)
        Xc[valid, :D] = x[sl[valid]]
        Xc[:, D] = 1.0
        XcT = Xc.T  # (36, SEG*Cc)
        js = chunk_judge[c * SEG : (c + 1) * SEG]
        # layout [a1 (H1) | xta (512) | xtc (W3)]
        l1in = np.zeros((SEG, 100, A1OFF + H1), np.float32)
        for s in range(SEG):
            base = s * Cc
            a1c = a1sb[js[s]]
            l1in[s, 0:D + 1, 0:H1] = a1c
            l1in[s, 64:64 + D + 1, 0:H1] = a1c
            l1in[s, 0:D + 1, H1:H1 + 512] = XcT[:, base : base + 512]
            l1in[s, 64:64 + D + 1, H1:H1 + 512] = XcT[:, base + 512 : base + 1024]
            l1in[s, 0:D + 1, H1 + 512:] = XcT[:, base + 1024 : base + Cc]
            l1in[s, 64:64 + D + 1, H1 + 512:] = XcT[:, base + 1024 : base + Cc]
        biasall = np.ascontiguousarray(
            np.transpose(bias3[js], (1, 0, 2)).reshape(128, 3 * SEG)
        )
        in_maps.append(
            {
                "l1in": l1in.astype(ml_dtypes.bfloat16),
                "a2av": np.ascontiguousarray(a2av[js]).astype(ml_dtypes.bfloat16),
                "biasall": biasall,
                "onesb": onesb,
                "onesf": onesf,
            }
        )
        core_meta.append((sl, valid))

    _patch_ldw_opt()
    nc = _get_program(Cc, USE_F32R)
    from concourse.bass_utils import run_bass_kernel_spmd

    res = run_bass_kernel_spmd(
        nc,
        in_maps,
        core_ids=list(range(NCORES)),
        trace=TRACE,
    )
    LAST_RESULTS = res

    full = np.zeros((Bx, Q, O), np.float32)
    for c in range(NCORES):
        oc = (
            np.asarray(res.results[c]["out"])
            .astype(np.float32)
            .reshape(108, SEG, 512)
        )
        sl, valid = core_meta[c]
        vals = np.empty((SEG * Cc, QO), np.float32)
        for s in range(SEG):
            for b in range(3):
                o, w = offs[b], widths[b]
                vals[s * Cc + o : s * Cc + o + w] = oc[QOp * b : QOp * b + QO, s, :w].T
        full[sl[valid]] = vals[valid].reshape(-1, Q, O)
    return full


# revision 53
# speedup vs baseline: 1.0343x; 1.0343x over previous
"""Trainium2 Bass kernel for nn_CalibrationNetwork (MoE routing over 12 judges).

Strategy: shared + judge-specific weights are pre-summed on the host into 12
effective per-judge MLPs; samples are sorted by judge id, each judge's slots
padded to capacity 2*Cc (Cc=1408, n-tiles 512/512/384 — sized to the actual
max judge count), and the 24 chunks dealt 3-per-core to 8 NeuronCores running
one static Bass/Tile program.

PE-array concurrency (tile_position row/col tiling) carries the slot count:
layer1 (K=36) runs as 3 row-tiled PAIRS — two matmuls with stationaries at
array rows 0-35 / 64-99 and moving operands at the matching partition rows
issue back-to-back and overlap in the array, so 6 matmuls cost 3 slots.  The
n2 x-block is duplicated at rows 0-35 and 64-99 so every pair is balanced.
The per-question heads (M=36) are col-tiled: n0's logits land at PSUM
partitions 0-35 (array cols 0-35) while n1's land at partitions 64-99 (cols
64-99), halving head slots.  Layer2 uses the full 128x128 array (no tiling).

Relu/bias evacuations are spread over ACT, DVE and GPSIMD so no single
engine exceeds the PE's ~4.1us/chunk.  Inputs arrive as two merged DMAs per
chunk (l1 inputs [100,1152] on sync, weights+heads [128,584] on gpsimd), and
each packed chunk's softmax output leaves as ONE [108,512] DMA against an
out DRAM layout of [108, SEG, 512] (row 36b+o = n-block b, question-out o).

Softmax head-major: heads accumulate a chunk's logits in a 2-bank PSUM tile;
exp(logit+bias) on ACT (pad rows get bias -1e30 -> e=0); the three 36-row
blocks are DMA-packed into one lane-dense [108,512] tile, then ONE group-sums
matmul, ONE reciprocal, ONE f32r cast, ONE broadcast matmul, ONE multiply.
The last chunk runs an unpacked tail (no pack-DMA latency): col-tiled n0/n1
plus the 384-col n2 split into two 192-col pieces, each piece a short
exp->sums->recip->cast->bcast->mult->DMA chain staggered across engines.

The PE clock gate (HAM) needs ~3.4us of busy to reach 2.4 GHz: 4 fp32
warmup matmuls (853ns each cold) bridge the initial DMA wait, and chunk s's
softmax matmuls are emitted between chunk s+1's layer2 m-phases to keep the
in-order PE queue dense.
"""

import os
import sys

import ml_dtypes
import numpy as np

for _p in ("/opt/trn_rl_repo", "/root/.axon_site/_ro/trn_rl_repo"):
    if os.path.isdir(_p) and _p not in sys.path:
        sys.path.insert(0, _p)

B, D, H1, H2, J, Q, O = 32768, 35, 256, 256, 12, 7, 5
NCORES = 8
SEG = 3                    # chunks per core
NCHUNKS = NCORES * SEG     # 24 = 2 chunks per judge
QO = Q * O                 # 35
QOp = QO + 1               # padded head dim

USE_F32R = True            # PE fast-fp32 mode (1 cyc/col vs 4 for fp32)
WARMUP_MM = 10             # fp32 warmup matmuls (fine-grained so the first
WARM_N = 128               # real matmul is never blocked behind a big one)
LDW_OPT = False            # walrus LDWEIGHTS optimizer (rejects col-tiled LDW)
TRACE = False
LAST_RESULTS = None

_PROG_CACHE = {}


def _patch_ldw_opt():
    """enable the LDWEIGHTS optimizer in walrus (overlaps/dedups stationary
    loads; measured ~11% end-to-end, bit-identical outputs)."""
    from concourse import bass_utils as BU

    if not LDW_OPT or getattr(BU, "_ldw_opt_patched", False):
        return
    orig = BU.run_command

    def patched(argv, **kw):
        argv = [
            "--enable-ldw-opt=true" if a == "--enable-ldw-opt=false" else a
            for a in argv
        ]
        return orig(argv, **kw)

    BU.run_command = patched
    BU._ldw_opt_patched = True


def _build_program(Cc, use_f32r):
    import contextlib

    import concourse.tile as tile
    from concourse import bacc, mybir

    f32 = mybir.dt.float32
    fmm = mybir.dt.float32r if use_f32r else f32
    bf16 = mybir.dt.bfloat16
    AF = mybir.ActivationFunctionType
    ALU = mybir.AluOpType

    W3 = Cc - 1024             # width of the third n-tile
    assert 128 <= W3 <= 512 and W3 % 64 == 0
    offs = (0, 512, 1024)
    widths = (512, 512, W3)
    A1OFF = 512 + W3           # a1 columns inside the merged l1in tile

    nc = bacc.Bacc(None, target_bir_lowering=False, debug=False, num_swdge_queues=2)

    l1in_d = nc.dram_tensor("l1in", [SEG, 100, A1OFF + H1], bf16, kind="ExternalInput")
    a2av_d = nc.dram_tensor("a2av", [SEG, 128, 512 + 2 * QOp], bf16, kind="ExternalInput")
    bias_d = nc.dram_tensor("biasall", [128, 3 * SEG], f32, kind="ExternalInput")
    # onesb columns: [0:24] packed sums, [24:132] packed bcast,
    # [132:140] single sums (rows 0-35), [140:176] single bcast (rows 0-7),
    # [176:184] single sums (rows 64-99), [184:312] ones (epk pad source)
    ones_d = nc.dram_tensor("onesb", [108, 312], fmm, kind="ExternalInput")
    # fp32 copies of the bcast matrices: the broadcast matmul runs in plain
    # fp32 so the reciprocal feeds it directly (no f32r rounding-copy op)
    onesf_d = nc.dram_tensor("onesf", [24, 144], f32, kind="ExternalInput")
    out_d = nc.dram_tensor("out", [108, SEG * 512], bf16, kind="ExternalOutput")

    lp = (
        nc.allow_low_precision(reason="float32r matmul operands are intentional")
        if use_f32r
        else contextlib.nullcontext()
    )
    with lp, tile.TileContext(nc) as tc:
        with (
            tc.tile_pool(name="xp", bufs=1) as xp,        # constants / warmup
            tc.tile_pool(name="inp", bufs=2) as inp,      # per-chunk inputs
            tc.tile_pool(name="zp", bufs=2) as zp,        # z1 / z2
            tc.tile_pool(name="op", bufs=2) as op_,       # softmax SBUF tiles
            tc.tile_pool(name="psM", bufs=5, space="PSUM") as psM,    # 5 banks
            tc.tile_pool(name="psH", bufs=1, space="PSUM") as psH,    # 2 banks
            tc.tile_pool(name="psS", bufs=1, space="PSUM") as psS,    # 1 bank
        ):
            onesb = xp.tile([108, 312], fmm)
            onesf = xp.tile([24, 144], f32)
            biasall = xp.tile([128, 3 * SEG], f32)
            ones_s = onesb[0:108, 0:24]
            ones_s1 = onesb[0:QOp, 132:140]
            ones_r = onesf[0:24, 0:108]
            ones_r1 = onesf[0:8, 108:144]

            def emit_load(s):
                h = {}
                # layout [a1 | xta | xtc]; two transfers so the first (a1+xta,
                # all three pair-A/C operands) lands before xtc is needed
                l1 = inp.tile([100, A1OFF + H1], bf16, tag="l1in", name=f"l1in_{s}")
                nc.sync.dma_start(l1[:, 0 : H1 + 512], l1in_d[s][:, 0 : H1 + 512])
                nc.sync.dma_start(l1[:, H1 + 512 :], l1in_d[s][:, H1 + 512 :])
                a2av = inp.tile([128, 512 + 2 * QOp], bf16, tag="a2av", name=f"a2av_{s}")
                nc.gpsimd.dma_start(a2av[:], a2av_d[s])
                h["a1"] = l1[:, 0:H1]
                h["xta"] = l1[:, H1 : H1 + 512]
                h["xtc"] = l1[:, H1 + 512 : H1 + 512 + W3]
                h["a2av"] = a2av
                h["z1"] = zp.tile([128, 2, Cc], bf16, tag="z1", name=f"z1_{s}")
                h["z2"] = zp.tile([128, 2, Cc], bf16, tag="z2", name=f"z2_{s}")
                return h

            def evac(eng, dst, src, bias=None):
                """PSUM->SBUF relu evacuation on ACT / DVE / GPSIMD."""
                if eng == "act":
                    if bias is None:
                        nc.scalar.activation(dst, src, AF.Relu)
                    else:
                        nc.scalar.activation(dst, src, AF.Relu, bias=bias)
                    return
                e = nc.vector if eng == "dve" else nc.gpsimd
                if bias is None:
                    e.tensor_scalar(
                        out=dst, in0=src, scalar1=0.0, scalar2=None, op0=ALU.max
                    )
                else:
                    e.tensor_scalar(
                        out=dst, in0=src, scalar1=bias, scalar2=0.0,
                        op0=ALU.add, op1=ALU.max,
                    )

            def emit_l1(s, h):
                """3 row-tiled pair slots: stationaries at array rows 0-35 and
                64-99 with moving operands at matching partitions overlap.
                Pair and evacuation-engine order matches layer2's consumption
                order (k0n0, k0n1, k1n0, k1n1, k0n2, k1n2) so each engine's
                FIFO delivers z1 halves exactly as layer2 needs them."""
                a1, xta, xtc, z1 = h["a1"], h["xta"], h["xtc"], h["z1"]
                pairs = [
                    # (w, lhsT_lo, rhs_lo, dst_lo, eng_lo, lhsT_hi, rhs_hi, dst_hi, eng_hi)
                    (512, a1[0:QOp, 0:128], xta[0:QOp, :], z1[:, 0, 0:512], "act",
                     a1[64:64 + QOp, 0:128], xta[64:64 + QOp, :], z1[:, 0, 512:1024], "dve"),
                    (512, a1[0:QOp, 128:256], xta[0:QOp, :], z1[:, 1, 0:512], "dve",
                     a1[64:64 + QOp, 128:256], xta[64:64 + QOp, :], z1[:, 1, 512:1024], "act"),
                    (W3, a1[0:QOp, 0:128], xtc[0:QOp, :], z1[:, 0, 1024:Cc], "act",
                     a1[64:64 + QOp, 128:256], xtc[64:64 + QOp, :], z1[:, 1, 1024:Cc], "dve"),
                ]
                for i, (w, llo, rlo, dlo, elo, lhi, rhi, dhi, ehi) in enumerate(pairs):
                    plo = psM.tile([128, 512], f32, tag="mm", name=f"p1_{s}{i}a")
                    phi = psM.tile([128, 512], f32, tag="mm", name=f"p1_{s}{i}b")
                    nc.tensor.matmul(plo[:, 0:w], llo, rlo, start=True, stop=True)
                    nc.tensor.matmul(phi[:, 0:w], lhi, rhi, start=True, stop=True)
                    evac(elo, dlo, plo[:, 0:w])
                    evac(ehi, dhi, phi[:, 0:w])

            # engine per (m, n): parallel ACT/DVE pairs in heads-consumption
            # order (heads k0 pair needs m0-n0/n1, k1 pair needs m1-n0/n1)
            L2_ENG = {(0, 0): "act", (0, 1): "dve", (0, 2): "act",
                      (1, 0): "dve", (1, 1): "act", (1, 2): "dve"}

            def emit_l2(s, h, m):
                """n0/n1 first (both k passes), n2 last — matches the arrival
                order of z1 evacuations and heads' consumption of z2."""
                a2av, z1, z2 = h["a2av"], h["z1"], h["z2"]
                bias = biasall[:, 3 * s + m : 3 * s + m + 1]
                st = [a2av[:, k * 256 + m * 128 : k * 256 + (m + 1) * 128]
                      for k in range(2)]
                p0 = psM.tile([128, 512], f32, tag="mm", name=f"p2_{s}{m}0")
                p1 = psM.tile([128, 512], f32, tag="mm", name=f"p2_{s}{m}1")
                nc.tensor.matmul(p0[:], st[0], z1[:, 0, 0:512], start=True, stop=False)
                nc.tensor.matmul(p1[:], st[0], z1[:, 0, 512:1024], start=True, stop=False)
                nc.tensor.matmul(p0[:], st[1], z1[:, 1, 0:512], start=False, stop=True)
                evac(L2_ENG[(m, 0)], z2[:, m, 0:512], p0[:], bias=bias)
                nc.tensor.matmul(p1[:], st[1], z1[:, 1, 512:1024], start=False, stop=True)
                evac(L2_ENG[(m, 1)], z2[:, m, 512:1024], p1[:], bias=bias)
                p2 = psM.tile([128, 512], f32, tag="mm", name=f"p2_{s}{m}2")
                nc.tensor.matmul(p2[:, 0:W3], st[0], z1[:, 0, 1024:Cc],
                                 start=True, stop=False)
                nc.tensor.matmul(p2[:, 0:W3], st[1], z1[:, 1, 1024:Cc],
                                 start=False, stop=True)
                evac(L2_ENG[(m, 2)], z2[:, m, 1024:Cc], p2[:, 0:W3], bias=bias)

            def emit_heads(s, h, last=False):
                """Col-tiled heads: n0 -> PSUM partitions 0-35 (array cols
                0-35), n1 -> partitions 64-99 (cols 64-99) — the two matmuls
                of each k overlap in the array.  n2 runs alone at (0,0)."""
                a2av, z2 = h["a2av"], h["z2"]
                blo = biasall[0:QOp, 3 * s + 2 : 3 * s + 3]
                bhi = biasall[64:64 + QOp, 3 * s + 2 : 3 * s + 3]
                if last:
                    # tail chunk: all logits at partitions 0-35 (f32r softmax
                    # matmuls cannot col-tile); n0+n2 share the psH tile so
                    # the tail's sums/bcast matmuls get the psM pool to
                    # themselves, n1 takes one psM tile
                    el = op_.tile([QOp, Cc], fmm, tag="el", name=f"el_{s}")
                    pha = psH.tile([100, 512 + W3], f32, tag="ph", name=f"phl_{s}a")
                    phb = psM.tile([QOp, 512], f32, tag="mm", name=f"phl_{s}b")
                    for k in range(2):
                        avk = a2av[:, 512 + k * QOp : 512 + (k + 1) * QOp]
                        nc.tensor.matmul(pha[0:QOp, 0:512], avk, z2[:, k, 0:512],
                                         start=(k == 0), stop=(k == 1))
                        nc.tensor.matmul(phb[0:QOp, 0:512], avk, z2[:, k, 512:1024],
                                         start=(k == 0), stop=(k == 1))
                        nc.tensor.matmul(pha[0:QOp, 512:512 + W3], avk,
                                         z2[:, k, 1024:Cc],
                                         start=(k == 0), stop=(k == 1))
                    wp = W3 // 2
                    nc.scalar.activation(el[:, 0:512], pha[0:QOp, 0:512],
                                         AF.Exp, bias=blo)
                    nc.scalar.activation(el[:, 512:1024], phb[0:QOp, 0:512],
                                         AF.Exp, bias=blo)
                    nc.scalar.activation(el[:, 1024:1024 + wp],
                                         pha[0:QOp, 512:512 + wp],
                                         AF.Exp, bias=blo)
                    nc.scalar.activation(el[:, 1024 + wp:Cc],
                                         pha[0:QOp, 512 + wp:512 + W3],
                                         AF.Exp, bias=blo)
                    return None, el, None
                ph = psH.tile([100, 512 + W3], f32, tag="ph", name=f"ph_{s}")
                # k0 pair, n2-k0, k1 pair, n2-k1: the n2-k0 matmul fills the
                # wait for z2[m1] evacuations before the k1 pair
                for k in range(2):
                    avk = a2av[:, 512 + k * QOp : 512 + (k + 1) * QOp]
                    nc.tensor.matmul(ph[0:QOp, 0:512], avk, z2[:, k, 0:512],
                                     start=(k == 0), stop=(k == 1))
                    nc.tensor.matmul(ph[64:64 + QOp, 0:512], avk, z2[:, k, 512:1024],
                                     start=(k == 0), stop=(k == 1))
                    nc.tensor.matmul(ph[0:QOp, 512:512 + W3], avk, z2[:, k, 1024:Cc],
                                     start=(k == 0), stop=(k == 1))
                e = op_.tile([100, 512 + W3], fmm, tag="e", name=f"e_{s}")
                epk = op_.tile([QOp * 3, 512], fmm, tag="epk", name=f"epk_{s}")
                nc.scalar.activation(epk[0:QOp, 0:512], ph[0:QOp, 0:512],
                                     AF.Exp, bias=blo)
                nc.scalar.activation(e[64:64 + QOp, 0:512], ph[64:64 + QOp, 0:512],
                                     AF.Exp, bias=bhi)
                nc.sync.dma_start(epk[QOp : 2 * QOp, 0:512], e[64:64 + QOp, 0:512])
                nc.scalar.activation(e[0:QOp, 512:512 + W3], ph[0:QOp, 512:512 + W3],
                                     AF.Exp, bias=blo)
                nc.gpsimd.dma_start(epk[2 * QOp : 3 * QOp, 0:W3],
                                    e[0:QOp, 512:512 + W3])
                if W3 < 512:
                    # fill the pad columns with 1.0: keeps their group sums
                    # nonzero (a 0-sum -> Inf recip -> 0*Inf NaN would pollute
                    # every row of those columns through the bcast matmul)
                    nc.gpsimd.dma_start(epk[2 * QOp : 3 * QOp, W3:512],
                                        ones_d[2 * QOp : 3 * QOp, 184 : 184 + 512 - W3])
                return ph, e, epk

            def emit_sm_sums(s, epk):
                sm = psS.tile([24, 512], f32, tag="smbc", name=f"sm_{s}")
                nc.tensor.matmul(sm[:], ones_s, epk[:], start=True, stop=True)
                rt = op_.tile([24, 512], f32, tag="rt", name=f"rt_{s}")
                nc.vector.reciprocal_approx_fast(rt[:], sm[:])
                return rt

            def emit_sm_fin(s, epk, rt):
                # broadcast matmul in plain fp32 (4 cyc/col): eats the f32r
                # rounding-copy op and feeds from the reciprocal directly
                bc = psS.tile([108, 512], f32, tag="smbc", name=f"bc_{s}")
                nc.tensor.matmul(bc[:], ones_r, rt[:], start=True, stop=True)
                outm = op_.tile([108, 512], bf16, tag="om", name=f"om_{s}")
                nc.vector.tensor_tensor(outm[:], epk[:], bc[:], ALU.mult)
                nc.sync.dma_start(out_d[:, s * 512 : (s + 1) * 512], outm[:])

            def emit_tail_last(s, el):
                """Unpacked per-piece softmax tail for the final chunk: four
                pieces (512/512/W3÷2/W3÷2), all at partitions 0-35."""
                wp = W3 // 2
                # pieces: (el col, width, out block row, out col)
                P = [
                    (0, 512, 0, 0),
                    (512, 512, QOp, 0),
                    (1024, wp, 2 * QOp, 0),
                    (1024 + wp, wp, 2 * QOp, wp),
                ]
                rt = op_.tile([8, Cc], f32, tag="rtl", name=f"rtl_{s}")
                outm = op_.tile([QOp, Cc], bf16, tag="oml", name=f"oml_{s}")
                for i, (ec, w, ob, oc) in enumerate(P):
                    sm = psM.tile([8, 512], f32, tag="mm", name=f"sml_{s}{i}")
                    nc.tensor.matmul(sm[:, 0:w], ones_s1, el[:, ec:ec + w],
                                     start=True, stop=True)
                    nc.vector.reciprocal_approx_fast(rt[:, ec:ec + w], sm[:, 0:w])
                for i, (ec, w, ob, oc) in enumerate(P):
                    bc = psM.tile([QOp, 512], f32, tag="mm", name=f"bcl_{s}{i}")
                    nc.tensor.matmul(bc[0:QOp, 0:w], ones_r1, rt[:, ec:ec + w],
                                     start=True, stop=True)
                    nc.vector.tensor_tensor(outm[:, ec:ec + w],
                                            el[:, ec:ec + w], bc[0:QOp, 0:w],
                                            ALU.mult)
                    deng = (nc.gpsimd, nc.sync)[i % 2]
                    deng.dma_start(out_d[ob:ob + QOp, s * 512 + oc : s * 512 + oc + w],
                                   outm[:, ec:ec + w])

            # ---- program body ----
            h0 = emit_load(0)
            nc.sync.dma_start(onesb[:], ones_d[:])
            nc.gpsimd.dma_start(biasall[:], bias_d[:])
            nc.gpsimd.dma_start(onesf[:], onesf_d[:])
            h1 = emit_load(1)

            # PE warmup: fp32 matmuls (4 cyc/col) bridge the initial DMA wait
            # and start the HAM activity window.
            wsrc = xp.tile([128, WARM_N], f32, tag="warmsrc")
            nc.vector.memset(wsrc[:], 0.0)
            wps = psS.tile([128, 512], f32, tag="smbc", name="warm_ps")
            for _ in range(WARMUP_MM):
                nc.tensor.matmul(wps[:, 0:WARM_N], wsrc[:, 0:128], wsrc[:],
                                 start=True, stop=True)

            # software pipeline: chunk s+1's layer1 is emitted BEFORE chunk
            # s's heads so the PE has independent work while z2(s)
            # evacuations drain; chunk s's softmax matmuls slot between
            # chunk s+1's layer2 m-phases.
            emit_l1(0, h0)
            emit_l2(0, h0, 0)
            emit_l2(0, h0, 1)
            h2 = emit_load(2)
            hs = [h0, h1, h2]
            emit_l1(1, h1)
            pend = (0, emit_heads(0, h0))
            for s in range(1, SEG):
                hn = hs[s]
                emit_l2(s, hn, 0)
                ps, (pph, pe, pepk) = pend
                rt = emit_sm_sums(ps, pepk)
                emit_l2(s, hn, 1)
                emit_sm_fin(ps, pepk, rt)
                if s < SEG - 1:
                    emit_l1(s + 1, hs[s + 1])
                pend = (s, emit_heads(s, hn, last=(s == SEG - 1)))
            ps, (pph, pe, pepk) = pend
            emit_tail_last(ps, pe)

    nc.compile()
    return nc


def _get_program(Cc, use_f32r):
    key = (Cc, use_f32r)
    if key not in _PROG_CACHE:
        _PROG_CACHE[key] = _build_program(Cc, use_f32r)
    return _PROG_CACHE[key]


def kernel(**inputs):
    global LAST_RESULTS
    x = np.ascontiguousarray(np.asarray(inputs["x"], dtype=np.float32))
    ids = np.asarray(inputs["judge_ids"]).astype(np.int64).ravel()
    W1_w = np.asarray(inputs["W1_w"], np.float32)
    W1_b = np.asarray(inputs["W1_b"], np.float32)
    W2_w = np.asarray(inputs["W2_w"], np.float32)
    W2_b = np.asarray(inputs["W2_b"], np.float32)
    W1a_w = np.asarray(inputs["W1a_w"], np.float32)
    W1a_b = np.asarray(inputs["W1a_b"], np.float32)
    W2a_w = np.asarray(inputs["W2a_w"], np.float32)
    W2a_b = np.asarray(inputs["W2a_b"], np.float32)
    V_w = np.asarray(inputs["V_w"], np.float32)
    V_b = np.asarray(inputs["V_b"], np.float32)
    Va_w = np.asarray(inputs["Va_w"], np.float32)
    Va_b = np.asarray(inputs["Va_b"], np.float32)

    Bx = x.shape[0]
    cnts = np.bincount(ids, minlength=J)
    Cc = 1408
    mx = int(cnts.max())
    while 2 * Cc < mx and Cc < 1536:
        Cc += 128
    assert 2 * Cc >= mx, "judge capacity exceeded"
    W3 = Cc - 1024
    offs = (0, 512, 1024)
    widths = (512, 512, W3)
    A1OFF = 512 + W3

    # effective per-judge weights (shared + judge-specific, biases folded)
    A1 = (W1_w[None] + W1a_w).copy()                      # (J, H1, D+1)
    A1[:, :, D] += W1_b[None] + W1a_b
    A2 = W2_w[None] + W2a_w                               # (J, H2, H1+1)
    b2 = A2[:, :, H1] + W2_b[None] + W2a_b                # (J, H2)
    A2c = A2[:, :, :H1]                                   # (J, H2, H1)
    AV = (V_w[None] + Va_w).reshape(J, QO, H2 + 1)
    bV = (AV[:, :, H2] + (V_b[None] + Va_b).reshape(J, QO)).astype(np.float32)
    AVc = AV[:, :, :H2]

    # SBUF layouts
    a1sb = np.ascontiguousarray(np.transpose(A1, (0, 2, 1)))  # (J, 36, 256)
    a2sb = np.transpose(A2c.reshape(J, H2, 2, 128), (0, 3, 2, 1))
    # (J, 128, 2, 256): [j,p,k,m] = A2c[j][m, k*128+p]
    avsb = np.transpose(AVc.reshape(J, QO, 2, 128), (0, 3, 2, 1))  # (J,128,2,35)
    avsb = np.concatenate(
        [avsb, np.zeros((J, 128, 2, QOp - QO), np.float32)], axis=3
    )
    a2av = np.concatenate(
        [
            np.ascontiguousarray(a2sb).reshape(J, 128, 512),
            np.ascontiguousarray(avsb).reshape(J, 128, 2 * QOp),
        ],
        axis=2,
    )  # (J, 128, 584)
    b2sb = np.ascontiguousarray(np.transpose(b2.reshape(J, 2, 128), (0, 2, 1)))

    # softmax bias column: rows 0-34 bV, rows 64-98 bV (for col-tiled n1),
    # all other rows -1e30 (pad -> e = 0)
    biasc = np.full((J, 128, 1), -1e30, np.float32)
    biasc[:, 0:QO, 0] = bV
    biasc[:, 64:64 + QO, 0] = bV
    bias3 = np.concatenate([b2sb, biasc], axis=2)  # (J, 128, 3)

    # ones matrices: packed (3-block) + single-block lo/hi + zero pad block
    onesb = np.zeros((108, 312), np.float32)
    for b in range(3):
        for o_ in range(QO):
            q = o_ // O
            onesb[QOp * b + o_, 8 * b + q] = 1.0
            onesb[8 * b + q, 24 + QOp * b + o_] = 1.0
        onesb[QOp * b : QOp * (b + 1), 8 * b + 7] = 1.0
    for o_ in range(QO):
        q = o_ // O
        onesb[o_, 132 + q] = 1.0
        onesb[q, 140 + o_] = 1.0
        onesb[64 + o_, 176 + q] = 1.0
    onesb[0:QOp, 132 + 7] = 1.0
    onesb[64:64 + QOp, 176 + 7] = 1.0
    onesb[:, 184:312] = 1.0          # epk pad fill (keeps pad sums nonzero)
    onesb = np.ascontiguousarray(onesb)
    # fp32 bcast matrices: [0:108] packed (24 rows), [108:144] single (8 rows)
    onesf = np.zeros((24, 144), np.float32)
    for b in range(3):
        for o_ in range(QO):
            onesf[8 * b + o_ // O, QOp * b + o_] = 1.0
    for o_ in range(QO):
        onesf[o_ // O, 108 + o_] = 1.0
    onesf = np.ascontiguousarray(onesf)

    # slot -> sample map: judge j owns slots [j*2Cc, (j+1)*2Cc)
    order = np.argsort(ids, kind="stable")
    slot2samp = np.full(NCHUNKS * Cc, -1, np.int64)
    pos = 0
    for j in range(J):
        k = int(cnts[j])
        slot2samp[j * 2 * Cc : j * 2 * Cc + k] = order[pos : pos + k]
        pos += k
    chunk_judge = np.repeat(np.arange(J), 2)

    in_maps = []
    core_meta = []
    for c in range(NCORES):
        sl = slot2samp[c * SEG * Cc : (c + 1) * SEG * Cc]
        valid = sl >= 0
        Xc = np.zeros((SEG * Cc, D + 1), np.float32)
        Xc[valid, :D] = x[sl[valid]]
        Xc[:, D] = 1.0
        XcT = Xc.T  # (36, SEG*Cc)
        js = chunk_judge[c * SEG : (c + 1) * SEG]
        # layout [a1 (H1) | xta (512) | xtc (W3)]
        l1in = np.zeros((SEG, 100, A1OFF + H1), np.float32)
        for s in range(SEG):
            base = s * Cc
            a1c = a1sb[js[s]]
            l1in[s, 0:D + 1, 0:H1] = a1c
            l1in[s, 64:64 + D + 1, 0:H1] = a1c
            l1in[s, 0:D + 1, H1:H1 + 512] = XcT[:, base : base + 512]
            l1in[s, 64:64 + D + 1, H1:H1 + 512] = XcT[:, base + 512 : base + 1024]
            l1in[s, 0:D + 1, H1 + 512:] = XcT[:, base + 1024 : base + Cc]
            l1in[s, 64:64 + D + 1, H1 + 512:] = XcT[:, base + 1024 : base + Cc]
        biasall = np.ascontiguousarray(
            np.transpose(bias3[js], (1, 0, 2)).reshape(128, 3 * SEG)
        )
        in_maps.append(
            {
                "l1in": l1in.astype(ml_dtypes.bfloat16),
                "a2av": np.ascontiguousarray(a2av[js]).astype(ml_dtypes.bfloat16),
                "biasall": biasall,
                "onesb": onesb,
                "onesf": onesf,
            }
        )
        core_meta.append((sl, valid))

    _patch_ldw_opt()
    nc = _get_program(Cc, USE_F32R)
    from concourse.bass_utils import run_bass_kernel_spmd

    res = run_bass_kernel_spmd(
        nc,
        in_maps,
        core_ids=list(range(NCORES)),
        trace=TRACE,
    )
    LAST_RESULTS = res

    full = np.zeros((Bx, Q, O), np.float32)
    for c in range(NCORES):
        oc = (
            np.asarray(res.results[c]["out"])
            .astype(np.float32)
            .reshape(108, SEG, 512)
        )
        sl, valid = core_meta[c]
        vals = np.empty((SEG * Cc, QO), np.float32)
        for s in range(SEG):
            for b in range(3):
                o, w = offs[b], widths[b]
                vals[s * Cc + o : s * Cc + o + w] = oc[QOp * b : QOp * b + QO, s, :w].T
        full[sl[valid]] = vals[valid].reshape(-1, Q, O)
    return full


# revision 57
# speedup vs baseline: 1.2040x; 1.1640x over previous
"""Trainium2 Bass kernel for nn_CalibrationNetwork (MoE routing over 12 judges).

Strategy: shared + judge-specific weights are pre-summed on the host into 12
effective per-judge MLPs; samples are sorted by judge id, each judge's slots
padded to capacity 2*Cc (Cc=1408, n-tiles 512/512/384 — sized to the actual
max judge count), and the 24 chunks dealt 3-per-core to 8 NeuronCores running
one static Bass/Tile program.

PE-array concurrency (tile_position row/col tiling) carries the slot count:
layer1 (K=36) runs as 3 row-tiled PAIRS — two matmuls with stationaries at
array rows 0-35 / 64-99 and moving operands at the matching partition rows
issue back-to-back and overlap in the array, so 6 matmuls cost 3 slots.  The
n2 x-block is duplicated at rows 0-35 and 64-99 so every pair is balanced.
The per-question heads (M=36) are col-tiled: n0's logits land at PSUM
partitions 0-35 (array cols 0-35) while n1's land at partitions 64-99 (cols
64-99), halving head slots.  Layer2 uses the full 128x128 array (no tiling).

Relu/bias evacuations are spread over ACT, DVE and GPSIMD so no single
engine exceeds the PE's ~4.1us/chunk.  Inputs arrive as two merged DMAs per
chunk (l1 inputs [100,1152] on sync, weights+heads [128,584] on gpsimd), and
each packed chunk's softmax output leaves as ONE [108,512] DMA against an
out DRAM layout of [108, SEG, 512] (row 36b+o = n-block b, question-out o).

Softmax head-major: heads accumulate a chunk's logits in a 2-bank PSUM tile;
exp(logit+bias) on ACT (pad rows get bias -1e30 -> e=0); the three 36-row
blocks are DMA-packed into one lane-dense [108,512] tile, then ONE group-sums
matmul, ONE reciprocal, ONE f32r cast, ONE broadcast matmul, ONE multiply.
The last chunk runs an unpacked tail (no pack-DMA latency): col-tiled n0/n1
plus the 384-col n2 split into two 192-col pieces, each piece a short
exp->sums->recip->cast->bcast->mult->DMA chain staggered across engines.

The PE clock gate (HAM) needs ~3.4us of busy to reach 2.4 GHz: 4 fp32
warmup matmuls (853ns each cold) bridge the initial DMA wait, and chunk s's
softmax matmuls are emitted between chunk s+1's layer2 m-phases to keep the
in-order PE queue dense.
"""

import os
import sys

import ml_dtypes
import numpy as np

for _p in ("/opt/trn_rl_repo", "/root/.axon_site/_ro/trn_rl_repo"):
    if os.path.isdir(_p) and _p not in sys.path:
        sys.path.insert(0, _p)

B, D, H1, H2, J, Q, O = 32768, 35, 256, 256, 12, 7, 5
NCORES = 8
SEG = 3                    # chunks per core
NCHUNKS = NCORES * SEG     # 24 = 2 chunks per judge
QO = Q * O                 # 35
QOp = QO + 1               # padded head dim

USE_F32R = True            # PE fast-fp32 mode (1 cyc/col vs 4 for fp32)
WARMUP_MM = 5              # fp32 warmup matmuls
WARM_N = 256
LDW_OPT = False            # walrus LDWEIGHTS optimizer (rejects col-tiled LDW)
TRACE = False
LAST_RESULTS = None

_PROG_CACHE = {}


def _patch_ldw_opt():
    """enable the LDWEIGHTS optimizer in walrus (overlaps/dedups stationary
    loads; measured ~11% end-to-end, bit-identical outputs)."""
    from concourse import bass_utils as BU

    if not LDW_OPT or getattr(BU, "_ldw_opt_patched", False):
        return
    orig = BU.run_command

    def patched(argv, **kw):
        argv = [
            "--enable-ldw-opt=true" if a == "--enable-ldw-opt=false" else a
            for a in argv
        ]
        return orig(argv, **kw)

    BU.run_command = patched
    BU._ldw_opt_patched = True


def _build_program(Cc, use_f32r):
    import contextlib

    import concourse.tile as tile
    from concourse import bacc, mybir

    f32 = mybir.dt.float32
    fmm = mybir.dt.float32r if use_f32r else f32
    bf16 = mybir.dt.bfloat16
    AF = mybir.ActivationFunctionType
    ALU = mybir.AluOpType

    W3 = Cc - 1024             # width of the third n-tile
    assert 128 <= W3 <= 512 and W3 % 64 == 0
    offs = (0, 512, 1024)
    widths = (512, 512, W3)
    A1OFF = 512 + W3           # a1 columns inside the merged l1in tile

    nc = bacc.Bacc(None, target_bir_lowering=False, debug=False, num_swdge_queues=2)

    l1in_d = nc.dram_tensor("l1in", [SEG, 100, A1OFF + H1], fmm, kind="ExternalInput")
    a2av_d = nc.dram_tensor("a2av", [SEG, 128, 512 + 2 * QOp], bf16, kind="ExternalInput")
    bias_d = nc.dram_tensor("biasall", [128, 3 * SEG], f32, kind="ExternalInput")
    # onesb columns: [0:24] packed sums, [24:132] packed bcast,
    # [132:140] single sums (rows 0-35), [140:176] single bcast (rows 0-7),
    # [176:184] single sums (rows 64-99), [184:312] ones (epk pad source)
    ones_d = nc.dram_tensor("onesb", [108, 312], fmm, kind="ExternalInput")
    # fp32 copies of the bcast matrices: the broadcast matmul runs in plain
    # fp32 so the reciprocal feeds it directly (no f32r rounding-copy op)
    onesf_d = nc.dram_tensor("onesf", [24, 144], f32, kind="ExternalInput")
    out_d = nc.dram_tensor("out", [108, SEG * 512], bf16, kind="ExternalOutput")

    lp = (
        nc.allow_low_precision(reason="float32r matmul operands are intentional")
        if use_f32r
        else contextlib.nullcontext()
    )
    with lp, tile.TileContext(nc) as tc:
        with (
            tc.tile_pool(name="xp", bufs=1) as xp,        # constants / warmup
            tc.tile_pool(name="inp", bufs=2) as inp,      # per-chunk inputs
            tc.tile_pool(name="zp", bufs=2) as zp,        # z1 / z2
            tc.tile_pool(name="op", bufs=2) as op_,       # softmax SBUF tiles
            tc.tile_pool(name="psM", bufs=5, space="PSUM") as psM,    # 5 banks
            tc.tile_pool(name="psH", bufs=1, space="PSUM") as psH,    # 2 banks
            tc.tile_pool(name="psS", bufs=1, space="PSUM") as psS,    # 1 bank
        ):
            onesb = xp.tile([108, 312], fmm)
            onesf = xp.tile([24, 144], f32)
            biasall = xp.tile([128, 3 * SEG], f32)
            ones_s = onesb[0:108, 0:24]
            ones_s1 = onesb[0:QOp, 132:140]
            ones_r = onesf[0:24, 0:108]
            ones_r1 = onesf[0:8, 108:144]

            def emit_load(s):
                h = {}
                # layout [a1 | xta | xtc]; two transfers so the first (a1+xta,
                # all three pair-A/C operands) lands before xtc is needed
                l1 = inp.tile([100, A1OFF + H1], fmm, tag="l1in", name=f"l1in_{s}")
                nc.sync.dma_start(l1[:, 0 : H1 + 512], l1in_d[s][:, 0 : H1 + 512])
                nc.sync.dma_start(l1[:, H1 + 512 :], l1in_d[s][:, H1 + 512 :])
                a2av = inp.tile([128, 512 + 2 * QOp], bf16, tag="a2av", name=f"a2av_{s}")
                nc.gpsimd.dma_start(a2av[:], a2av_d[s])
                h["a1"] = l1[:, 0:H1]
                h["xta"] = l1[:, H1 : H1 + 512]
                h["xtc"] = l1[:, H1 + 512 : H1 + 512 + W3]
                h["a2av"] = a2av
                h["z1"] = zp.tile([128, 2, Cc], bf16, tag="z1", name=f"z1_{s}")
                h["z2"] = zp.tile([128, 2, Cc], bf16, tag="z2", name=f"z2_{s}")
                return h

            def evac(eng, dst, src, bias=None):
                """PSUM->SBUF relu evacuation on ACT / DVE / GPSIMD."""
                if eng == "act":
                    if bias is None:
                        nc.scalar.activation(dst, src, AF.Relu)
                    else:
                        nc.scalar.activation(dst, src, AF.Relu, bias=bias)
                    return
                e = nc.vector if eng == "dve" else nc.gpsimd
                if bias is None:
                    e.tensor_scalar(
                        out=dst, in0=src, scalar1=0.0, scalar2=None, op0=ALU.max
                    )
                else:
                    e.tensor_scalar(
                        out=dst, in0=src, scalar1=bias, scalar2=0.0,
                        op0=ALU.add, op1=ALU.max,
                    )

            def emit_l1(s, h):
                """3 row-tiled pair slots: stationaries at array rows 0-35 and
                64-99 with moving operands at matching partitions overlap.
                Pair and evacuation-engine order matches layer2's consumption
                order (k0n0, k0n1, k1n0, k1n1, k0n2, k1n2) so each engine's
                FIFO delivers z1 halves exactly as layer2 needs them."""
                a1, xta, xtc, z1 = h["a1"], h["xta"], h["xtc"], h["z1"]
                pairs = [
                    # (w, lhsT_lo, rhs_lo, dst_lo, eng_lo, lhsT_hi, rhs_hi, dst_hi, eng_hi)
                    (512, a1[0:QOp, 0:128], xta[0:QOp, :], z1[:, 0, 0:512], "act",
                     a1[64:64 + QOp, 0:128], xta[64:64 + QOp, :], z1[:, 0, 512:1024], "dve"),
                    (512, a1[0:QOp, 128:256], xta[0:QOp, :], z1[:, 1, 0:512], "dve",
                     a1[64:64 + QOp, 128:256], xta[64:64 + QOp, :], z1[:, 1, 512:1024], "act"),
                    (W3, a1[0:QOp, 0:128], xtc[0:QOp, :], z1[:, 0, 1024:Cc], "act",
                     a1[64:64 + QOp, 128:256], xtc[64:64 + QOp, :], z1[:, 1, 1024:Cc], "dve"),
                ]
                for i, (w, llo, rlo, dlo, elo, lhi, rhi, dhi, ehi) in enumerate(pairs):
                    plo = psM.tile([128, 512], f32, tag="mm", name=f"p1_{s}{i}a")
                    phi = psM.tile([128, 512], f32, tag="mm", name=f"p1_{s}{i}b")
                    nc.tensor.matmul(plo[:, 0:w], llo, rlo, start=True, stop=True)
                    nc.tensor.matmul(phi[:, 0:w], lhi, rhi, start=True, stop=True)
                    evac(elo, dlo, plo[:, 0:w])
                    evac(ehi, dhi, phi[:, 0:w])

            # engine per (m, n): parallel ACT/DVE pairs in heads-consumption
            # order (heads k0 pair needs m0-n0/n1, k1 pair needs m1-n0/n1)
            L2_ENG = {(0, 0): "act", (0, 1): "dve", (0, 2): "act",
                      (1, 0): "dve", (1, 1): "act", (1, 2): "dve"}

            def emit_l2(s, h, m):
                """n0/n1 first (both k passes), n2 last — matches the arrival
                order of z1 evacuations and heads' consumption of z2."""
                a2av, z1, z2 = h["a2av"], h["z1"], h["z2"]
                bias = biasall[:, 3 * s + m : 3 * s + m + 1]
                st = [a2av[:, k * 256 + m * 128 : k * 256 + (m + 1) * 128]
                      for k in range(2)]
                p0 = psM.tile([128, 512], f32, tag="mm", name=f"p2_{s}{m}0")
                p1 = psM.tile([128, 512], f32, tag="mm", name=f"p2_{s}{m}1")
                nc.tensor.matmul(p0[:], st[0], z1[:, 0, 0:512], start=True, stop=False)
                nc.tensor.matmul(p1[:], st[0], z1[:, 0, 512:1024], start=True, stop=False)
                nc.tensor.matmul(p0[:], st[1], z1[:, 1, 0:512], start=False, stop=True)
                evac(L2_ENG[(m, 0)], z2[:, m, 0:512], p0[:], bias=bias)
                nc.tensor.matmul(p1[:], st[1], z1[:, 1, 512:1024], start=False, stop=True)
                evac(L2_ENG[(m, 1)], z2[:, m, 512:1024], p1[:], bias=bias)
                p2 = psM.tile([128, 512], f32, tag="mm", name=f"p2_{s}{m}2")
                nc.tensor.matmul(p2[:, 0:W3], st[0], z1[:, 0, 1024:Cc],
                                 start=True, stop=False)
                nc.tensor.matmul(p2[:, 0:W3], st[1], z1[:, 1, 1024:Cc],
                                 start=False, stop=True)
                evac(L2_ENG[(m, 2)], z2[:, m, 1024:Cc], p2[:, 0:W3], bias=bias)

            def emit_heads(s, h, last=False):
                """Col-tiled heads: n0 -> PSUM partitions 0-35 (array cols
                0-35), n1 -> partitions 64-99 (cols 64-99) — the two matmuls
                of each k overlap in the array.  n2 runs alone at (0,0)."""
                a2av, z2 = h["a2av"], h["z2"]
                blo = biasall[0:QOp, 3 * s + 2 : 3 * s + 3]
                bhi = biasall[64:64 + QOp, 3 * s + 2 : 3 * s + 3]
                if last:
                    # tail chunk: all logits at partitions 0-35 (f32r softmax
                    # matmuls cannot col-tile); n0+n2 share the psH tile so
                    # the tail's sums/bcast matmuls get the psM pool to
                    # themselves, n1 takes one psM tile
                    el = op_.tile([QOp, Cc], fmm, tag="el", name=f"el_{s}")
                    pha = psH.tile([100, 512 + W3], f32, tag="ph", name=f"phl_{s}a")
                    phb = psM.tile([QOp, 512], f32, tag="mm", name=f"phl_{s}b")
                    for k in range(2):
                        avk = a2av[:, 512 + k * QOp : 512 + (k + 1) * QOp]
                        nc.tensor.matmul(pha[0:QOp, 0:512], avk, z2[:, k, 0:512],
                                         start=(k == 0), stop=(k == 1))
                        nc.tensor.matmul(phb[0:QOp, 0:512], avk, z2[:, k, 512:1024],
                                         start=(k == 0), stop=(k == 1))
                        nc.tensor.matmul(pha[0:QOp, 512:512 + W3], avk,
                                         z2[:, k, 1024:Cc],
                                         start=(k == 0), stop=(k == 1))
                    wp = W3 // 2
                    nc.scalar.activation(el[:, 0:512], pha[0:QOp, 0:512],
                                         AF.Exp, bias=blo)
                    nc.scalar.activation(el[:, 512:1024], phb[0:QOp, 0:512],
                                         AF.Exp, bias=blo)
                    nc.scalar.activation(el[:, 1024:1024 + wp],
                                         pha[0:QOp, 512:512 + wp],
                                         AF.Exp, bias=blo)
                    nc.scalar.activation(el[:, 1024 + wp:Cc],
                                         pha[0:QOp, 512 + wp:512 + W3],
                                         AF.Exp, bias=blo)
                    return None, el, None
                ph = psH.tile([100, 512 + W3], f32, tag="ph", name=f"ph_{s}")
                # k0 pair, n2-k0, k1 pair, n2-k1: the n2-k0 matmul fills the
                # wait for z2[m1] evacuations before the k1 pair
                for k in range(2):
                    avk = a2av[:, 512 + k * QOp : 512 + (k + 1) * QOp]
                    nc.tensor.matmul(ph[0:QOp, 0:512], avk, z2[:, k, 0:512],
                                     start=(k == 0), stop=(k == 1))
                    nc.tensor.matmul(ph[64:64 + QOp, 0:512], avk, z2[:, k, 512:1024],
                                     start=(k == 0), stop=(k == 1))
                    nc.tensor.matmul(ph[0:QOp, 512:512 + W3], avk, z2[:, k, 1024:Cc],
                                     start=(k == 0), stop=(k == 1))
                e = op_.tile([100, 512 + W3], fmm, tag="e", name=f"e_{s}")
                epk = op_.tile([QOp * 3, 512], fmm, tag="epk", name=f"epk_{s}")
                nc.scalar.activation(epk[0:QOp, 0:512], ph[0:QOp, 0:512],
                                     AF.Exp, bias=blo)
                nc.scalar.activation(e[64:64 + QOp, 0:512], ph[64:64 + QOp, 0:512],
                                     AF.Exp, bias=bhi)
                nc.sync.dma_start(epk[QOp : 2 * QOp, 0:512], e[64:64 + QOp, 0:512])
                nc.scalar.activation(e[0:QOp, 512:512 + W3], ph[0:QOp, 512:512 + W3],
                                     AF.Exp, bias=blo)
                nc.gpsimd.dma_start(epk[2 * QOp : 3 * QOp, 0:W3],
                                    e[0:QOp, 512:512 + W3])
                if W3 < 512:
                    # fill the pad columns with 1.0: keeps their group sums
                    # nonzero (a 0-sum -> Inf recip -> 0*Inf NaN would pollute
                    # every row of those columns through the bcast matmul)
                    nc.gpsimd.dma_start(epk[2 * QOp : 3 * QOp, W3:512],
                                        ones_d[2 * QOp : 3 * QOp, 184 : 184 + 512 - W3])
                return ph, e, epk

            def emit_sm_sums(s, epk):
                sm = psS.tile([24, 512], f32, tag="smbc", name=f"sm_{s}")
                nc.tensor.matmul(sm[:], ones_s, epk[:], start=True, stop=True)
                rt = op_.tile([24, 512], f32, tag="rt", name=f"rt_{s}")
                nc.vector.reciprocal_approx_fast(rt[:], sm[:])
                return rt

            def emit_sm_fin(s, epk, rt):
                # broadcast matmul in plain fp32 (4 cyc/col): eats the f32r
                # rounding-copy op and feeds from the reciprocal directly
                bc = psS.tile([108, 512], f32, tag="smbc", name=f"bc_{s}")
                nc.tensor.matmul(bc[:], ones_r, rt[:], start=True, stop=True)
                outm = op_.tile([108, 512], bf16, tag="om", name=f"om_{s}")
                nc.vector.tensor_tensor(outm[:], epk[:], bc[:], ALU.mult)
                nc.sync.dma_start(out_d[:, s * 512 : (s + 1) * 512], outm[:])

            def emit_tail_last(s, el):
                """Unpacked per-piece softmax tail for the final chunk: four
                pieces (512/512/W3÷2/W3÷2), all at partitions 0-35."""
                wp = W3 // 2
                # pieces: (el col, width, out block row, out col)
                P = [
                    (0, 512, 0, 0),
                    (512, 512, QOp, 0),
                    (1024, wp, 2 * QOp, 0),
                    (1024 + wp, wp, 2 * QOp, wp),
                ]
                rt = op_.tile([8, Cc], f32, tag="rtl", name=f"rtl_{s}")
                outm = op_.tile([QOp, Cc], bf16, tag="oml", name=f"oml_{s}")
                for i, (ec, w, ob, oc) in enumerate(P):
                    sm = psM.tile([8, 512], f32, tag="mm", name=f"sml_{s}{i}")
                    nc.tensor.matmul(sm[:, 0:w], ones_s1, el[:, ec:ec + w],
                                     start=True, stop=True)
                    nc.vector.reciprocal_approx_fast(rt[:, ec:ec + w], sm[:, 0:w])
                for i, (ec, w, ob, oc) in enumerate(P):
                    bc = psM.tile([QOp, 512], f32, tag="mm", name=f"bcl_{s}{i}")
                    nc.tensor.matmul(bc[0:QOp, 0:w], ones_r1, rt[:, ec:ec + w],
                                     start=True, stop=True)
                    nc.vector.tensor_tensor(outm[:, ec:ec + w],
                                            el[:, ec:ec + w], bc[0:QOp, 0:w],
                                            ALU.mult)
                    deng = (nc.gpsimd, nc.sync)[i % 2]
                    deng.dma_start(out_d[ob:ob + QOp, s * 512 + oc : s * 512 + oc + w],
                                   outm[:, ec:ec + w])

            # ---- program body ----
            h0 = emit_load(0)
            nc.sync.dma_start(onesb[:], ones_d[:])
            nc.gpsimd.dma_start(biasall[:], bias_d[:])
            nc.gpsimd.dma_start(onesf[:], onesf_d[:])
            h1 = emit_load(1)

            # PE warmup: fp32 matmuls (4 cyc/col) bridge the initial DMA wait
            # and start the HAM activity window.
            wsrc = xp.tile([128, WARM_N], f32, tag="warmsrc")
            nc.vector.memset(wsrc[:], 0.0)
            wps = psS.tile([128, 512], f32, tag="smbc", name="warm_ps")
            for _ in range(WARMUP_MM):
                nc.tensor.matmul(wps[:, 0:WARM_N], wsrc[:, 0:128], wsrc[:],
                                 start=True, stop=True)

            # software pipeline: chunk s+1's layer1 is emitted BEFORE chunk
            # s's heads so the PE has independent work while z2(s)
            # evacuations drain; chunk s's softmax matmuls slot between
            # chunk s+1's layer2 m-phases.
            emit_l1(0, h0)
            emit_l2(0, h0, 0)
            emit_l2(0, h0, 1)
            h2 = emit_load(2)
            hs = [h0, h1, h2]
            emit_l1(1, h1)
            pend = (0, emit_heads(0, h0))
            for s in range(1, SEG):
                hn = hs[s]
                emit_l2(s, hn, 0)
                ps, (pph, pe, pepk) = pend
                rt = emit_sm_sums(ps, pepk)
                emit_l2(s, hn, 1)
                emit_sm_fin(ps, pepk, rt)
                if s < SEG - 1:
                    emit_l1(s + 1, hs[s + 1])
                pend = (s, emit_heads(s, hn, last=(s == SEG - 1)))
            ps, (pph, pe, pepk) = pend
            emit_tail_last(ps, pe)

    nc.compile()
    return nc


def _get_program(Cc, use_f32r):
    key = (Cc, use_f32r)
    if key not in _PROG_CACHE:
        _PROG_CACHE[key] = _build_program(Cc, use_f32r)
    return _PROG_CACHE[key]


def kernel(**inputs):
    global LAST_RESULTS
    x = np.ascontiguousarray(np.asarray(inputs["x"], dtype=np.float32))
    ids = np.asarray(inputs["judge_ids"]).astype(np.int64).ravel()
    W1_w = np.asarray(inputs["W1_w"], np.float32)
    W1_b = np.asarray(inputs["W1_b"], np.float32)
    W2_w = np.asarray(inputs["W2_w"], np.float32)
    W2_b = np.asarray(inputs["W2_b"], np.float32)
    W1a_w = np.asarray(inputs["W1a_w"], np.float32)
    W1a_b = np.asarray(inputs["W1a_b"], np.float32)
    W2a_w = np.asarray(inputs["W2a_w"], np.float32)
    W2a_b = np.asarray(inputs["W2a_b"], np.float32)
    V_w = np.asarray(inputs["V_w"], np.float32)
    V_b = np.asarray(inputs["V_b"], np.float32)
    Va_w = np.asarray(inputs["Va_w"], np.float32)
    Va_b = np.asarray(inputs["Va_b"], np.float32)

    Bx = x.shape[0]
    cnts = np.bincount(ids, minlength=J)
    Cc = 1408
    mx = int(cnts.max())
    while 2 * Cc < mx and Cc < 1536:
        Cc += 128
    assert 2 * Cc >= mx, "judge capacity exceeded"
    W3 = Cc - 1024
    offs = (0, 512, 1024)
    widths = (512, 512, W3)
    A1OFF = 512 + W3

    # effective per-judge weights (shared + judge-specific, biases folded)
    A1 = (W1_w[None] + W1a_w).copy()                      # (J, H1, D+1)
    A1[:, :, D] += W1_b[None] + W1a_b
    A2 = W2_w[None] + W2a_w                               # (J, H2, H1+1)
    b2 = A2[:, :, H1] + W2_b[None] + W2a_b                # (J, H2)
    A2c = A2[:, :, :H1]                                   # (J, H2, H1)
    AV = (V_w[None] + Va_w).reshape(J, QO, H2 + 1)
    bV = (AV[:, :, H2] + (V_b[None] + Va_b).reshape(J, QO)).astype(np.float32)
    AVc = AV[:, :, :H2]

    # SBUF layouts
    a1sb = np.ascontiguousarray(np.transpose(A1, (0, 2, 1)))  # (J, 36, 256)
    a2sb = np.transpose(A2c.reshape(J, H2, 2, 128), (0, 3, 2, 1))
    # (J, 128, 2, 256): [j,p,k,m] = A2c[j][m, k*128+p]
    avsb = np.transpose(AVc.reshape(J, QO, 2, 128), (0, 3, 2, 1))  # (J,128,2,35)
    avsb = np.concatenate(
        [avsb, np.zeros((J, 128, 2, QOp - QO), np.float32)], axis=3
    )
    a2av = np.concatenate(
        [
            np.ascontiguousarray(a2sb).reshape(J, 128, 512),
            np.ascontiguousarray(avsb).reshape(J, 128, 2 * QOp),
        ],
        axis=2,
    )  # (J, 128, 584)
    b2sb = np.ascontiguousarray(np.transpose(b2.reshape(J, 2, 128), (0, 2, 1)))

    # softmax bias column: rows 0-34 bV, rows 64-98 bV (for col-tiled n1),
    # all other rows -1e30 (pad -> e = 0)
    biasc = np.full((J, 128, 1), -1e30, np.float32)
    biasc[:, 0:QO, 0] = bV
    biasc[:, 64:64 + QO, 0] = bV
    bias3 = np.concatenate([b2sb, biasc], axis=2)  # (J, 128, 3)

    # ones matrices: packed (3-block) + single-block lo/hi + zero pad block
    onesb = np.zeros((108, 312), np.float32)
    for b in range(3):
        for o_ in range(QO):
            q = o_ // O
            onesb[QOp * b + o_, 8 * b + q] = 1.0
            onesb[8 * b + q, 24 + QOp * b + o_] = 1.0
        onesb[QOp * b : QOp * (b + 1), 8 * b + 7] = 1.0
    for o_ in range(QO):
        q = o_ // O
        onesb[o_, 132 + q] = 1.0
        onesb[q, 140 + o_] = 1.0
        onesb[64 + o_, 176 + q] = 1.0
    onesb[0:QOp, 132 + 7] = 1.0
    onesb[64:64 + QOp, 176 + 7] = 1.0
    onesb[:, 184:312] = 1.0          # epk pad fill (keeps pad sums nonzero)
    onesb = np.ascontiguousarray(onesb)
    # fp32 bcast matrices: [0:108] packed (24 rows), [108:144] single (8 rows)
    onesf = np.zeros((24, 144), np.float32)
    for b in range(3):
        for o_ in range(QO):
            onesf[8 * b + o_ // O, QOp * b + o_] = 1.0
    for o_ in range(QO):
        onesf[o_ // O, 108 + o_] = 1.0
    onesf = np.ascontiguousarray(onesf)

    # slot -> sample map: judge j owns slots [j*2Cc, (j+1)*2Cc)
    order = np.argsort(ids, kind="stable")
    slot2samp = np.full(NCHUNKS * Cc, -1, np.int64)
    pos = 0
    for j in range(J):
        k = int(cnts[j])
        slot2samp[j * 2 * Cc : j * 2 * Cc + k] = order[pos : pos + k]
        pos += k
    chunk_judge = np.repeat(np.arange(J), 2)

    in_maps = []
    core_meta = []
    for c in range(NCORES):
        sl = slot2samp[c * SEG * Cc : (c + 1) * SEG * Cc]
        valid = sl >= 0
        Xc = np.zeros((SEG * Cc, D + 1), np.float32)
        Xc[valid, :D] = x[sl[valid]]
        Xc[:, D] = 1.0
        XcT = Xc.T  # (36, SEG*Cc)
        js = chunk_judge[c * SEG : (c + 1) * SEG]
        # layout [a1 (H1) | xta (512) | xtc (W3)]
        l1in = np.zeros((SEG, 100, A1OFF + H1), np.float32)
        for s in range(SEG):
            base = s * Cc
            a1c = a1sb[js[s]]
            l1in[s, 0:D + 1, 0:H1] = a1c
            l1in[s, 64:64 + D + 1, 0:H1] = a1c
            l1in[s, 0:D + 1, H1:H1 + 512] = XcT[:, base : base + 512]
            l1in[s, 64:64 + D + 1, H1:H1 + 512] = XcT[:, base + 512 : base + 1024]
            l1in[s, 0:D + 1, H1 + 512:] = XcT[:, base + 1024 : base + Cc]
            l1in[s, 64:64 + D + 1, H1 + 512:] = XcT[:, base + 1024 : base + Cc]
        biasall = np.ascontiguousarray(
            np.transpose(bias3[js], (1, 0, 2)).reshape(128, 3 * SEG)
        )
        in_maps.append(
            {
                "l1in": l1in,
                "a2av": np.ascontiguousarray(a2av[js]).astype(ml_dtypes.bfloat16),
                "biasall": biasall,
                "onesb": onesb,
                "onesf": onesf,
            }
        )
        core_meta.append((sl, valid))

    _patch_ldw_opt()
    nc = _get_program(Cc, USE_F32R)
    from concourse.bass_utils import run_bass_kernel_spmd

    res = run_bass_kernel_spmd(
        nc,
        in_maps,
        core_ids=list(range(NCORES)),
        trace=TRACE,
    )
    LAST_RESULTS = res

    full = np.zeros((Bx, Q, O), np.float32)
    for c in range(NCORES):
        oc = (
            np.asarray(res.results[c]["out"])
            .astype(np.float32)
            .reshape(108, SEG, 512)
        )
        sl, valid = core_meta[c]
        vals = np.empty((SEG * Cc, QO), np.float32)
        for s in range(SEG):
            for b in range(3):
                o, w = offs[b], widths[b]
                vals[s * Cc + o : s * Cc + o + w] = oc[QOp * b : QOp * b + QO, s, :w].T
        full[sl[valid]] = vals[valid].reshape(-1, Q, O)
    return full
